# revision 21
# baseline (speedup 1.0000x reference)
"""Causal single-head attention on 8 TRN2 NeuronCores.

Problem (hardcoded): x [4, 2048, 1024] f32; Wk, Wq, Wv [1024, 1024] f32.
  q = x @ Wk.T ; k = x @ Wq.T ; v = x @ Wv.T        (note ref's q/k weight swap)
  out = softmax(mask(q @ k.T) / sqrt(1024)) @ v

Sharding: 2 cores per batch.  Core h of a batch owns four 256-query strips,
processed in "slots" with a fixed causal context template {512, 1024, 1536,
2048} keys: h=0 owns strips {0,3,4,7} (q0 = 0, 768, 1024, 1792), h=1 owns
{1,2,5,6} (256, 512, 1280, 1536) — every core runs the identical program
(true SPMD); the slot template dominates each core's per-strip causal needs
and per-core additive masks (only the last 4 key-chunks of each slot can be
non-trivial: diagonal or template padding) encode causality exactly.

K/V projection is FULLY split across the pair: core h projects K^T and V
only for its own 1024 keys (global keys [h*1024:(h+1)*1024)) and the halves
are exchanged through DRAM bounce buffers with three pair AllGathers (K in
two 512-key pieces for early availability, V in one) — collective_compute
blocks its engine until completion, so the ops serialize on the CC stream;
triggers are arranged so the chain still lands each tensor before first
use.  Own-key projection output is staged in the low half of the K^T / V
SBUF tensors; the gather readback (both regions, so the final key order is
global and identical on both ranks) overwrites them.

On-chip layout is feature-major (all host-side transposes are free):
  xT/wT in, Q^T/K^T feature-major, V sequence-major.  Scores are computed
  as S^T[k, q] so softmax needs no on-chip transpose anywhere: the score
  loop runs key-chunk-outer so one KT slice load feeds up to four slots'
  matmuls; exp via ACT (no max subtraction -- scaled scores are ~N(0,1)),
  sum-of-exp via a ones-column matmul per slot as soon as its chunks are
  done, AV runs e-outer/key-inner so one V slice load feeds up to four
  slots, accumulating out^T[e, q].  The per-query 1/sum is broadcast
  across partitions with a K=1 PE matmul and applied by DVE during the
  PSUM->SBUF output copy.  Output returns as out^T in slot order and is
  scattered back on the host.  All matmuls bf16 with fp32 PSUM
  accumulation.

The kernel opens with warm-up matmuls on a zeroed tile so the PE HAM
clock-gate reaches 8/8 (2.4 GHz) while the first input DMAs are in flight;
the K projection runs d-outer so its first matmul only needs ~0.4 MB of
DMA.  DMA queues: streaming loads and exchange DMAs all go on the sync
queue, emitted in the order their semaphore waits resolve (a blocked DMA
head-of-line-blocks its queue).
"""

import functools

import ml_dtypes
import numpy as np

B = 4
S = 2048
D = 1024
P = 128
DCH = D // P            # 8 contraction chunks
QT = 512                # projection column-tile width
QS = 256                # query-strip width (phase 2)
KO = 1024               # own keys per core (projection split)
NKB = S // P            # 16 key chunks
TPL = (4, 8, 12, 16)    # context template per slot, in 128-key chunks
NEG = np.float32(-30000.0)
WARMUP_MM = 18

_BF16 = ml_dtypes.bfloat16

# slot -> strip start q0, per h  (strip req <= 128*TPL[slot])
_QSTARTS = ((0, 768, 1024, 1792), (256, 512, 1280, 1536))


@functools.lru_cache(maxsize=1)
def _build_nc():
    import concourse.bass as bass  # noqa: F401  (registers engines)
    import concourse.mybir as mybir
    from concourse import bacc, tile

    bf16 = mybir.dt.bfloat16
    f32 = mybir.dt.float32
    add = mybir.AluOpType.add
    mult = mybir.AluOpType.mult
    Exp = mybir.ActivationFunctionType.Exp
    PAIRS = [[2 * i, 2 * i + 1] for i in range(4)]

    nc = bacc.Bacc("TRN2", target_bir_lowering=False, debug=False, num_devices=8)

    xT = nc.declare_dram_parameter("xT", [D, KO], bf16, isOutput=False)
    xqT = nc.declare_dram_parameter("xqT", [D, 4 * QS], bf16, isOutput=False)
    wqT = nc.declare_dram_parameter("wqT", [D, D], bf16, isOutput=False)
    wkT = nc.declare_dram_parameter("wkT", [D, D], bf16, isOutput=False)
    wvT = nc.declare_dram_parameter("wvT", [D, D], bf16, isOutput=False)
    masks = nc.declare_dram_parameter("masks", [16 * P, QS], bf16,
                                      isOutput=False)
    outT = nc.declare_dram_parameter("outT", [D, 4 * QS], f32, isOutput=True)

    with tile.TileContext(nc) as tc:
        with (
            tc.tile_pool(name="kv", bufs=1) as kv,
            tc.tile_pool(name="dram", bufs=1, space="DRAM") as dram,
        ):
            # ---- persistent SBUF tensors --------------------------------
            kt_sb = [kv.tile([P, S], bf16, tag=f"kt{e}", name=f"kt{e}")
                     for e in range(DCH)]
            qt_sb = [kv.tile([P, 4 * QS], bf16, tag=f"qt{e}", name=f"qt{e}")
                     for e in range(DCH)]
            v_sb = [kv.tile([P, D], bf16, tag=f"v{t}", name=f"v{t}")
                    for t in range(NKB)]
            ones_sb = kv.tile([P, 1], bf16, tag="ones", name="ones")
            nc.gpsimd.memset(ones_sb[:], 1.0)
            onesr = kv.tile([1, P], f32, tag="onesr", name="onesr")
            nc.gpsimd.memset(onesr[:], 1.0)
            # touch the Exp LUT once so the lazy activation-table load isn't
            # on the first score tile's critical path
            scr = kv.tile([P, 1], f32, tag="scr", name="scr")
            nc.scalar.activation(scr[:], ones_sb[:], Exp)
            # mask tiles: slot s, local chunk j (= key chunk TPL[s]-4+j)
            msk_sb = [[kv.tile([P, QS], bf16, tag=f"m{s}_{j}",
                               name=f"m{s}_{j}") for j in range(4)]
                      for s in range(4)]

            # DRAM bounce buffers for the pair K/V exchange.  K goes in two
            # 512-key pieces so early key chunks land early; V in one (the
            # blocking CC ops serialize anyway and V is needed last).
            # Region r of each agout holds group-rank r's piece.
            agin_k = [dram.tile([D, QT], bf16, name=f"agin_k{i}")
                      for i in range(2)]
            agout_k = [dram.tile([2 * D, QT], bf16, name=f"agout_k{i}")
                       for i in range(2)]
            agin_v = dram.tile([8 * P, D], bf16, name="agin_v")
            agout_v = dram.tile([16 * P, D], bf16, name="agout_v")

            # ---- phase 1: load inputs + QKV projections -----------------
            with (
                tc.tile_pool(name="inp", bufs=1) as inp,
                tc.tile_pool(name="pps", bufs=2, space="PSUM") as pps,
            ):
                x_sb = [inp.tile([P, KO], bf16, tag=f"x{d}", name=f"x{d}")
                        for d in range(DCH)]
                xq_sb = [inp.tile([P, 4 * QS], bf16, tag=f"xq{d}",
                                  name=f"xq{d}") for d in range(DCH)]
                wq_sb = [inp.tile([P, D], bf16, tag=f"wq{d}", name=f"wq{d}")
                         for d in range(DCH)]
                wk_sb = [inp.tile([P, D], bf16, tag=f"wk{d}", name=f"wk{d}")
                         for d in range(DCH)]
                wv_sb = [inp.tile([P, D], bf16, tag=f"wv{d}", name=f"wv{d}")
                        for d in range(DCH)]
                warm = inp.tile([P, QT], bf16, tag="warm", name="warm")

                # PE warm-up: HAM un-throttles after ~3.4us of sustained
                # matmul activity; burn the initial DMA window on junk
                # matmuls so the real ones run at 2.4 GHz from the start.
                nc.gpsimd.memset(warm[:], 0.0)
                wps = pps.tile([P, QT], f32, tag="pj0", name="wps")
                for i in range(WARMUP_MM):
                    nc.tensor.matmul(wps[:], warm[:, 0:P], warm[:],
                                     start=True, stop=True,
                                     skip_group_check=True)

                # Streaming loads in first-use order on the sync queue.  K
                # projection runs d-outer, so interleave x (key-half 0)
                # with wk per d-chunk: the first matmul group only needs
                # ~0.4 MB.  wv and later loads are emitted after
                # k_proj_half(0) so agin_k0 enters the queue early.
                for d in range(DCH):
                    rows = slice(d * P, (d + 1) * P)
                    nc.sync.dma_start(out=x_sb[d][:, 0:QT],
                                      in_=xT[rows, 0:QT])
                    nc.sync.dma_start(out=wk_sb[d][:], in_=wkT[rows, :])
                for d in range(DCH):
                    rows = slice(d * P, (d + 1) * P)
                    nc.sync.dma_start(out=x_sb[d][:, QT:KO],
                                      in_=xT[rows, QT:KO])

                # K^T for the 1024 own keys, half (512 keys) at a time so
                # each half's AllGather fires as early as possible.  Loop
                # d-outer with 4-wide e-groups accumulating in parallel
                # PSUM banks; the first group only waits on x[d0]/wk[d0].
                def k_proj_half(half):
                    hsl = slice(half * QT, (half + 1) * QT)
                    for eg in range(2):
                        es = range(eg * 4, eg * 4 + 4)
                        pss = [pps.tile([P, QT], f32, tag=f"pj{i}",
                                        name=f"kps{half}_{eg}_{i}")
                               for i in range(4)]
                        for d in range(DCH):
                            for i, e in enumerate(es):
                                nc.tensor.matmul(
                                    pss[i][:],
                                    wk_sb[d][:, e * P:(e + 1) * P],
                                    x_sb[d][:, hsl],
                                    start=(d == 0), stop=(d == DCH - 1),
                                    skip_group_check=True,
                                )
                        for i, e in enumerate(es):
                            nc.vector.tensor_copy(kt_sb[e][:, hsl], pss[i][:])
                    for e in range(DCH):
                        nc.sync.dma_start(out=agin_k[half][e * P:(e + 1) * P, :],
                                          in_=kt_sb[e][:, hsl])
                    nc.gpsimd.collective_compute(
                        "AllGather", mybir.AluOpType.bypass,
                        replica_groups=PAIRS,
                        ins=[agin_k[half][:]], outs=[agout_k[half][:]],
                    )

                # Read back BOTH regions of a gather: final key order is
                # global and identical on both ranks.  half h, region r ->
                # kt cols [r*1024 + h*512 : .. + 512).
                def k_readback(half):
                    for r in range(2):
                        csl = slice(r * KO + half * QT, r * KO + (half + 1) * QT)
                        for e in range(DCH):
                            nc.sync.dma_start(
                                out=kt_sb[e][:, csl],
                                in_=agout_k[half][r * D + e * P:
                                                  r * D + (e + 1) * P, :])

                def v_proj(t):
                    tsl = slice(t * P, (t + 1) * P)
                    pss = [pps.tile([P, QT], f32, tag=f"pj{eh}",
                                    name=f"vps{t}_{eh}") for eh in range(2)]
                    for d in range(DCH):
                        for eh in range(2):
                            nc.tensor.matmul(
                                pss[eh][:], x_sb[d][:, tsl],
                                wv_sb[d][:, eh * QT:(eh + 1) * QT],
                                start=(d == 0), stop=(d == DCH - 1),
                            )
                    for eh in range(2):
                        nc.vector.tensor_copy(
                            v_sb[t][:, eh * QT:(eh + 1) * QT], pss[eh][:])

                # Emission order staggers the sync-queue DMAs so each one's
                # semaphore wait resolves roughly when the queue reaches it
                # (a blocked DMA stalls everything behind it on its queue).
                k_proj_half(0)
                for d in range(DCH):
                    rows = slice(d * P, (d + 1) * P)
                    nc.sync.dma_start(out=wv_sb[d][:], in_=wvT[rows, :])
                for d in range(DCH):
                    rows = slice(d * P, (d + 1) * P)
                    nc.sync.dma_start(out=wq_sb[d][:], in_=wqT[rows, :])
                    nc.sync.dma_start(out=xq_sb[d][:], in_=xqT[rows, :])
                for s in range(4):
                    for j in range(4):
                        rows = slice((4 * s + j) * P, (4 * s + j + 1) * P)
                        nc.sync.dma_start(out=msk_sb[s][j][:],
                                          in_=masks[rows, :])
                k_proj_half(1)
                k_readback(0)
                for t in range(DCH):
                    v_proj(t)
                for t in range(DCH):
                    nc.sync.dma_start(out=agin_v[t * P:(t + 1) * P, :],
                                      in_=v_sb[t][:])
                nc.gpsimd.collective_compute(
                    "AllGather", mybir.AluOpType.bypass,
                    replica_groups=PAIRS,
                    ins=[agin_v[:]], outs=[agout_v[:]],
                )
                k_readback(1)

                # Q^T[e, q]: one wq weight tile drives two query halves.
                for e in range(DCH):
                    esl = slice(e * P, (e + 1) * P)
                    pss = [pps.tile([P, QT], f32, tag=f"pj{qh}",
                                    name=f"qps{e}_{qh}") for qh in range(2)]
                    for d in range(DCH):
                        for qh in range(2):
                            nc.tensor.matmul(
                                pss[qh][:], wq_sb[d][:, esl],
                                xq_sb[d][:, qh * QT:(qh + 1) * QT],
                                start=(d == 0), stop=(d == DCH - 1),
                            )
                    for qh in range(2):
                        nc.vector.tensor_copy(
                            qt_sb[e][:, qh * QT:(qh + 1) * QT], pss[qh][:])

                # V readback: region r chunk i -> v_sb[r*8 + i]
                for r in range(2):
                    for i in range(DCH):
                        nc.sync.dma_start(
                            out=v_sb[r * 8 + i][:],
                            in_=agout_v[(r * 8 + i) * P:(r * 8 + i + 1) * P, :])

            # ---- phase 2: attention over four 256-query slots -----------
            # Slots are processed in PAIRS (0,1) and (2,3): for key chunks
            # both slots of a pair need, ONE N=512 matmul covers both
            # (their queries are adjacent in qt).  A pair shares PSUM
            # banks; the upper slot's private chunks continue accumulating
            # into the bank's high half with start=False (has_written bits
            # are already set -- start=True would clear the WHOLE bank and
            # wipe the partner's partials).
            with (
                tc.tile_pool(name="pp", bufs=1) as pp,
                tc.tile_pool(name="ost", bufs=4) as ost,
                tc.tile_pool(name="msc", bufs=2) as msc,
                tc.tile_pool(name="scp", bufs=3, space="PSUM") as scp,
                tc.tile_pool(name="sol", bufs=1, space="PSUM") as sol,
                tc.tile_pool(name="smp", bufs=1, space="PSUM") as smp,
            ):
                TLO = (TPL[0], TPL[2])   # pair g: low slot = 2g, high = 2g+1
                THI = (TPL[1], TPL[3])
                phat = {}   # (g, k) -> [P, 2*QS] (k < TLO) or [P, QS]
                recb = {}   # pair -> [P, 2*QS] f32 broadcast reciprocal

                def pair_sum(g):
                    sps = smp.tile([1, 2 * QS], f32, tag="sm", name=f"sum{g}")
                    for k in range(THI[g]):
                        if k < TLO[g]:
                            nc.tensor.matmul(
                                sps[:], ones_sb[:, 0:1], phat[g, k][:],
                                start=(k == 0), stop=False,
                                skip_group_check=True,
                            )
                        else:
                            nc.tensor.matmul(
                                sps[:, QS:2 * QS], ones_sb[:, 0:1],
                                phat[g, k][:],
                                start=False, stop=(k == THI[g] - 1),
                                skip_group_check=True,
                            )
                    srow = msc.tile([1, 2 * QS], f32, tag="srow", name=f"srow{g}")
                    nc.vector.tensor_copy(srow[:], sps[:])
                    bc = smp.tile([P, 2 * QS], f32, tag="sm", name=f"bc{g}")
                    nc.tensor.matmul(bc[:], onesr[:, 0:P], srow[:],
                                     start=True, stop=True)
                    rb = msc.tile([P, 2 * QS], f32, tag=f"recb{g}",
                                  name=f"recb{g}")
                    nc.vector.reciprocal_approx_fast(out=rb[:], in_=bc[:])
                    recb[g] = rb

                # scores, key-chunk-outer: one kt slice load drives both
                # pairs' matmuls.  Only the last 4 chunks of each slot can
                # need masking (diagonal or template padding); for the low
                # slot those fall in the pair phase (mask the low half),
                # for the high slot in its solo phase.
                for k in range(NKB):
                    ksl = slice(k * P, (k + 1) * P)
                    ps = {}
                    for g in range(2):
                        if k < TLO[g]:
                            ps[g] = scp.tile([P, 2 * QS], f32, tag=f"pg{g}",
                                             name=f"pg{g}_{k}")
                        elif k < THI[g]:
                            ps[g] = sol.tile([P, QS], f32, tag="so",
                                             name=f"so{g}_{k}")
                    for e in range(DCH):
                        for g in range(2):
                            if k < TLO[g]:
                                nc.tensor.matmul(
                                    ps[g][:], kt_sb[e][:, ksl],
                                    qt_sb[e][:, 2 * g * QS:(2 * g + 2) * QS],
                                    start=(e == 0), stop=(e == DCH - 1),
                                    skip_group_check=True,
                                )
                            elif k < THI[g]:
                                nc.tensor.matmul(
                                    ps[g][:], kt_sb[e][:, ksl],
                                    qt_sb[e][:, (2 * g + 1) * QS:
                                               (2 * g + 2) * QS],
                                    start=(e == 0), stop=(e == DCH - 1),
                                    skip_group_check=True,
                                )
                    for g in range(2):
                        if k < TLO[g]:
                            j = k - (TLO[g] - 4)
                            if j >= 0:
                                nc.vector.tensor_tensor(
                                    ps[g][:, 0:QS], ps[g][:, 0:QS],
                                    msk_sb[2 * g][j][:], op=add)
                            ph = pp.tile([P, 2 * QS], bf16, tag=f"pp{g}_{k}",
                                         name=f"php{g}_{k}")
                            nc.scalar.activation(ph[:], ps[g][:], Exp,
                                                 scale=0.03125)
                            phat[g, k] = ph
                        elif k < THI[g]:
                            j = k - (THI[g] - 4)
                            if j >= 0:
                                nc.vector.tensor_tensor(
                                    ps[g][:], ps[g][:],
                                    msk_sb[2 * g + 1][j][:], op=add)
                            ph = pp.tile([P, QS], bf16, tag=f"po{g}_{k}",
                                         name=f"pho{g}_{k}")
                            nc.scalar.activation(ph[:], ps[g][:], Exp,
                                                 scale=0.03125)
                            phat[g, k] = ph
                        if k == THI[g] - 1:
                            pair_sum(g)

                # AV, e-outer / key-chunk-inner: one V slice load drives
                # both pairs.  out^T[e, q], two slots per PSUM bank.
                for e in range(DCH):
                    esl = slice(e * P, (e + 1) * P)
                    av = [scp.tile([P, 2 * QS], f32, tag=f"pg{g}",
                                   name=f"avg{g}_{e}") for g in range(2)]
                    for k in range(NKB):
                        for g in range(2):
                            if k < TLO[g]:
                                nc.tensor.matmul(
                                    av[g][:], v_sb[k][:, esl], phat[g, k][:],
                                    start=(k == 0), stop=False,
                                    skip_group_check=True,
                                )
                            elif k < THI[g]:
                                nc.tensor.matmul(
                                    av[g][:, QS:2 * QS], v_sb[k][:, esl],
                                    phat[g, k][:],
                                    start=False, stop=(k == THI[g] - 1),
                                    skip_group_check=True,
                                )
                    for g in range(2):
                        ot = ost.tile([P, 2 * QS], f32, tag="ot",
                                      name=f"ot{g}_{e}")
                        nc.vector.tensor_tensor(ot[:], av[g][:], recb[g][:],
                                                op=mult)
                        nc.sync.dma_start(
                            out=outT[esl, 2 * g * QS:(2 * g + 2) * QS],
                            in_=ot[:])

    nc.compile()
    return nc


def _make_masks(h: int) -> np.ndarray:
    """[16*128, 256] bf16: slot s rows [4s*128:(4s+4)*128) = key chunks
    TPL[s]-4 .. TPL[s]-1 vs that slot's 256 queries."""
    m = np.empty((16 * P, QS), dtype=np.float32)
    for s in range(4):
        q0 = _QSTARTS[h][s]
        q = q0 + np.arange(QS)[None, :]
        for j in range(4):
            c = TPL[s] - 4 + j
            kk = c * P + np.arange(P)[:, None]
            m[(4 * s + j) * P:(4 * s + j + 1) * P] = np.where(kk <= q, 0.0, NEG)
    return m.astype(_BF16)


def _in_maps(x, Wk, Wq, Wv):
    wq_t = np.ascontiguousarray(Wk.T.astype(_BF16))   # ref swap: q uses Wk
    wk_t = np.ascontiguousarray(Wq.T.astype(_BF16))
    wv_t = np.ascontiguousarray(Wv.T.astype(_BF16))
    mby_h = [_make_masks(0), _make_masks(1)]
    maps = []
    for c in range(8):
        b, h = divmod(c, 2)
        xb = x[b].astype(_BF16)
        # own keys for the K/V projection split
        x_t = np.ascontiguousarray(xb[h * KO:(h + 1) * KO].T)
        xq_t = np.ascontiguousarray(
            np.concatenate([xb[q0:q0 + QS] for q0 in _QSTARTS[h]], axis=0).T
        )
        maps.append({
            "xT": x_t,
            "xqT": xq_t,
            "wqT": wq_t,
            "wkT": wk_t,
            "wvT": wv_t,
            "masks": mby_h[h],
        })
    return maps


def _assemble(results):
    out = np.empty((B, S, D), dtype=np.float32)
    for c, res in enumerate(results):
        b, h = divmod(c, 2)
        o = res["outT"]
        for s, q0 in enumerate(_QSTARTS[h]):
            out[b, q0:q0 + QS] = o[:, s * QS:(s + 1) * QS].T
    return out


def kernel(x, Wk, Wq, Wv, _trace=False):
    from concourse.bass_utils import run_bass_kernel_spmd

    nc = _build_nc()
    res = run_bass_kernel_spmd(nc, _in_maps(x, Wk, Wq, Wv), list(range(8)),
                               trace=_trace)
    out = _assemble(res.results)
    if _trace:
        return out, res
    return out


# revision 22
# speedup vs baseline: 1.0183x; 1.0183x over previous
"""Causal single-head attention on 8 TRN2 NeuronCores.

Problem (hardcoded): x [4, 2048, 1024] f32; Wk, Wq, Wv [1024, 1024] f32.
  q = x @ Wk.T ; k = x @ Wq.T ; v = x @ Wv.T        (note ref's q/k weight swap)
  out = softmax(mask(q @ k.T) / sqrt(1024)) @ v

Sharding: 2 cores per batch.  Core h of a batch owns four 256-query strips,
processed in "slots" with a fixed causal context template {512, 1024, 1536,
2048} keys: h=0 owns strips {0,3,4,7} (q0 = 0, 768, 1024, 1792), h=1 owns
{1,2,5,6} (256, 512, 1280, 1536) — every core runs the identical program
(true SPMD); the slot template dominates each core's per-strip causal needs
and per-core additive masks (only the last 4 key-chunks of each slot can be
non-trivial: diagonal or template padding) encode causality exactly.

K/V projection is FULLY split across the pair: core h projects K^T and V
only for its own 1024 keys (global keys [h*1024:(h+1)*1024)) and the halves
are exchanged through DRAM bounce buffers with three pair AllGathers (K in
two 512-key pieces for early availability, V in one) — collective_compute
blocks its engine until completion, so the ops serialize on the CC stream;
triggers are arranged so the chain still lands each tensor before first
use.  Own-key projection output is staged in the low half of the K^T / V
SBUF tensors; the gather readback (both regions, so the final key order is
global and identical on both ranks) overwrites them.

On-chip layout is feature-major (all host-side transposes are free):
  xT/wT in, Q^T/K^T feature-major, V sequence-major.  Scores are computed
  as S^T[k, q] so softmax needs no on-chip transpose anywhere: the score
  loop runs key-chunk-outer so one KT slice load feeds up to four slots'
  matmuls; exp via ACT (no max subtraction -- scaled scores are ~N(0,1)),
  sum-of-exp via a ones-column matmul per slot as soon as its chunks are
  done, AV runs e-outer/key-inner so one V slice load feeds up to four
  slots, accumulating out^T[e, q].  The per-query 1/sum is broadcast
  across partitions with a K=1 PE matmul and applied by DVE during the
  PSUM->SBUF output copy.  Output returns as out^T in slot order and is
  scattered back on the host.  All matmuls bf16 with fp32 PSUM
  accumulation.

The kernel opens with warm-up matmuls on a zeroed tile so the PE HAM
clock-gate reaches 8/8 (2.4 GHz) while the first input DMAs are in flight;
the K projection runs d-outer so its first matmul only needs ~0.4 MB of
DMA.  DMA queues: streaming loads and exchange DMAs all go on the sync
queue, emitted in the order their semaphore waits resolve (a blocked DMA
head-of-line-blocks its queue).
"""

import functools

import ml_dtypes
import numpy as np

B = 4
S = 2048
D = 1024
P = 128
DCH = D // P            # 8 contraction chunks
QT = 512                # projection column-tile width
QS = 256                # query-strip width (phase 2)
KO = 1024               # own keys per core (projection split)
NKB = S // P            # 16 key chunks
TPL = (4, 8, 12, 16)    # context template per slot, in 128-key chunks
NEG = np.float32(-30000.0)
WARMUP_MM = 18

_BF16 = ml_dtypes.bfloat16

# slot -> strip start q0, per h  (strip req <= 128*TPL[slot])
_QSTARTS = ((0, 768, 1024, 1792), (256, 512, 1280, 1536))


@functools.lru_cache(maxsize=1)
def _build_nc():
    import concourse.bass as bass  # noqa: F401  (registers engines)
    import concourse.mybir as mybir
    from concourse import bacc, tile

    bf16 = mybir.dt.bfloat16
    f32 = mybir.dt.float32
    add = mybir.AluOpType.add
    mult = mybir.AluOpType.mult
    Exp = mybir.ActivationFunctionType.Exp
    PAIRS = [[2 * i, 2 * i + 1] for i in range(4)]

    nc = bacc.Bacc("TRN2", target_bir_lowering=False, debug=False, num_devices=8)

    xT = nc.declare_dram_parameter("xT", [D, KO], bf16, isOutput=False)
    xqT = nc.declare_dram_parameter("xqT", [D, 4 * QS], bf16, isOutput=False)
    wqT = nc.declare_dram_parameter("wqT", [D, D], bf16, isOutput=False)
    wkT = nc.declare_dram_parameter("wkT", [D, D], bf16, isOutput=False)
    wvT = nc.declare_dram_parameter("wvT", [D, D], bf16, isOutput=False)
    masks = nc.declare_dram_parameter("masks", [16 * P, QS], bf16,
                                      isOutput=False)
    outT = nc.declare_dram_parameter("outT", [D, 4 * QS], f32, isOutput=True)

    with tile.TileContext(nc) as tc:
        with (
            tc.tile_pool(name="kv", bufs=1) as kv,
            tc.tile_pool(name="dram", bufs=1, space="DRAM") as dram,
        ):
            # ---- persistent SBUF tensors --------------------------------
            kt_sb = [kv.tile([P, S], bf16, tag=f"kt{e}", name=f"kt{e}")
                     for e in range(DCH)]
            qt_sb = [kv.tile([P, 4 * QS], bf16, tag=f"qt{e}", name=f"qt{e}")
                     for e in range(DCH)]
            v_sb = [kv.tile([P, D], bf16, tag=f"v{t}", name=f"v{t}")
                    for t in range(NKB)]
            ones_sb = kv.tile([P, 1], bf16, tag="ones", name="ones")
            nc.gpsimd.memset(ones_sb[:], 1.0)
            onesr = kv.tile([1, P], f32, tag="onesr", name="onesr")
            nc.gpsimd.memset(onesr[:], 1.0)
            # touch the Exp LUT once so the lazy activation-table load isn't
            # on the first score tile's critical path
            scr = kv.tile([P, 1], f32, tag="scr", name="scr")
            nc.scalar.activation(scr[:], ones_sb[:], Exp)
            # mask tiles: slot s, local chunk j (= key chunk TPL[s]-4+j)
            msk_sb = [[kv.tile([P, QS], bf16, tag=f"m{s}_{j}",
                               name=f"m{s}_{j}") for j in range(4)]
                      for s in range(4)]

            # DRAM bounce buffers for the pair K/V exchange.  K goes in two
            # 512-key pieces so early key chunks land early; V in one (the
            # blocking CC ops serialize anyway and V is needed last).
            # Region r of each agout holds group-rank r's piece.
            agin_k = [dram.tile([D, QT], bf16, name=f"agin_k{i}")
                      for i in range(2)]
            agout_k = [dram.tile([2 * D, QT], bf16, name=f"agout_k{i}")
                       for i in range(2)]
            agin_v = dram.tile([8 * P, D], bf16, name="agin_v")
            agout_v = dram.tile([16 * P, D], bf16, name="agout_v")

            # ---- phase 1: load inputs + QKV projections -----------------
            with (
                tc.tile_pool(name="inp", bufs=1) as inp,
                tc.tile_pool(name="pps", bufs=2, space="PSUM") as pps,
            ):
                x_sb = [inp.tile([P, KO], bf16, tag=f"x{d}", name=f"x{d}")
                        for d in range(DCH)]
                xq_sb = [inp.tile([P, 4 * QS], bf16, tag=f"xq{d}",
                                  name=f"xq{d}") for d in range(DCH)]
                wq_sb = [inp.tile([P, D], bf16, tag=f"wq{d}", name=f"wq{d}")
                         for d in range(DCH)]
                wk_sb = [inp.tile([P, D], bf16, tag=f"wk{d}", name=f"wk{d}")
                         for d in range(DCH)]
                wv_sb = [inp.tile([P, D], bf16, tag=f"wv{d}", name=f"wv{d}")
                        for d in range(DCH)]
                warm = inp.tile([P, QT], bf16, tag="warm", name="warm")

                # PE warm-up: HAM un-throttles after ~3.4us of sustained
                # matmul activity; burn the initial DMA window on junk
                # matmuls so the real ones run at 2.4 GHz from the start.
                nc.gpsimd.memset(warm[:], 0.0)
                wps = pps.tile([P, QT], f32, tag="pj0", name="wps")
                for i in range(WARMUP_MM):
                    nc.tensor.matmul(wps[:], warm[:, 0:P], warm[:],
                                     start=True, stop=True,
                                     skip_group_check=True)

                # Streaming loads in first-use order on the sync queue.  K
                # projection runs d-outer, so interleave x (key-half 0)
                # with wk per d-chunk: the first matmul group only needs
                # ~0.4 MB.  wv and later loads are emitted after
                # k_proj_half(0) so agin_k0 enters the queue early.
                for d in range(DCH):
                    rows = slice(d * P, (d + 1) * P)
                    nc.sync.dma_start(out=x_sb[d][:, 0:QT],
                                      in_=xT[rows, 0:QT])
                    nc.sync.dma_start(out=wk_sb[d][:], in_=wkT[rows, :])
                for d in range(DCH):
                    rows = slice(d * P, (d + 1) * P)
                    nc.sync.dma_start(out=x_sb[d][:, QT:KO],
                                      in_=xT[rows, QT:KO])

                # K^T for the 1024 own keys, half (512 keys) at a time so
                # each half's AllGather fires as early as possible.  Loop
                # d-outer with 4-wide e-groups accumulating in parallel
                # PSUM banks; the first group only waits on x[d0]/wk[d0].
                def k_proj_half(half):
                    hsl = slice(half * QT, (half + 1) * QT)
                    for eg in range(2):
                        es = range(eg * 4, eg * 4 + 4)
                        pss = [pps.tile([P, QT], f32, tag=f"pj{i}",
                                        name=f"kps{half}_{eg}_{i}")
                               for i in range(4)]
                        for d in range(DCH):
                            for i, e in enumerate(es):
                                nc.tensor.matmul(
                                    pss[i][:],
                                    wk_sb[d][:, e * P:(e + 1) * P],
                                    x_sb[d][:, hsl],
                                    start=(d == 0), stop=(d == DCH - 1),
                                    skip_group_check=True,
                                )
                        for i, e in enumerate(es):
                            nc.vector.tensor_copy(kt_sb[e][:, hsl], pss[i][:])
                    for e in range(DCH):
                        nc.sync.dma_start(out=agin_k[half][e * P:(e + 1) * P, :],
                                          in_=kt_sb[e][:, hsl])
                    nc.gpsimd.collective_compute(
                        "AllGather", mybir.AluOpType.bypass,
                        replica_groups=PAIRS,
                        ins=[agin_k[half][:]], outs=[agout_k[half][:]],
                    )

                # Read back BOTH regions of a gather: final key order is
                # global and identical on both ranks.  half h, region r ->
                # kt cols [r*1024 + h*512 : .. + 512).
                def k_readback(half):
                    for r in range(2):
                        csl = slice(r * KO + half * QT, r * KO + (half + 1) * QT)
                        for e in range(DCH):
                            nc.sync.dma_start(
                                out=kt_sb[e][:, csl],
                                in_=agout_k[half][r * D + e * P:
                                                  r * D + (e + 1) * P, :])

                def v_proj(t):
                    tsl = slice(t * P, (t + 1) * P)
                    pss = [pps.tile([P, QT], f32, tag=f"pj{eh}",
                                    name=f"vps{t}_{eh}") for eh in range(2)]
                    for d in range(DCH):
                        for eh in range(2):
                            nc.tensor.matmul(
                                pss[eh][:], x_sb[d][:, tsl],
                                wv_sb[d][:, eh * QT:(eh + 1) * QT],
                                start=(d == 0), stop=(d == DCH - 1),
                            )
                    for eh in range(2):
                        nc.vector.tensor_copy(
                            v_sb[t][:, eh * QT:(eh + 1) * QT], pss[eh][:])

                # Emission order staggers the sync-queue DMAs so each one's
                # semaphore wait resolves roughly when the queue reaches it
                # (a blocked DMA stalls everything behind it on its queue).
                k_proj_half(0)
                for d in range(DCH):
                    rows = slice(d * P, (d + 1) * P)
                    nc.sync.dma_start(out=wv_sb[d][:], in_=wvT[rows, :])
                for d in range(DCH):
                    rows = slice(d * P, (d + 1) * P)
                    nc.sync.dma_start(out=wq_sb[d][:], in_=wqT[rows, :])
                    nc.sync.dma_start(out=xq_sb[d][:], in_=xqT[rows, :])
                for s in range(4):
                    for j in range(4):
                        rows = slice((4 * s + j) * P, (4 * s + j + 1) * P)
                        nc.sync.dma_start(out=msk_sb[s][j][:],
                                          in_=masks[rows, :])
                k_proj_half(1)
                k_readback(0)
                for t in range(DCH):
                    v_proj(t)
                for t in range(DCH):
                    nc.sync.dma_start(out=agin_v[t * P:(t + 1) * P, :],
                                      in_=v_sb[t][:])
                nc.gpsimd.collective_compute(
                    "AllGather", mybir.AluOpType.bypass,
                    replica_groups=PAIRS,
                    ins=[agin_v[:]], outs=[agout_v[:]],
                )
                k_readback(1)

                # Q^T[e, q]: one wq weight tile drives two query halves.
                for e in range(DCH):
                    esl = slice(e * P, (e + 1) * P)
                    pss = [pps.tile([P, QT], f32, tag=f"pj{qh}",
                                    name=f"qps{e}_{qh}") for qh in range(2)]
                    for d in range(DCH):
                        for qh in range(2):
                            nc.tensor.matmul(
                                pss[qh][:], wq_sb[d][:, esl],
                                xq_sb[d][:, qh * QT:(qh + 1) * QT],
                                start=(d == 0), stop=(d == DCH - 1),
                            )
                    for qh in range(2):
                        nc.vector.tensor_copy(
                            qt_sb[e][:, qh * QT:(qh + 1) * QT], pss[qh][:])

                # V readback: region r chunk i -> v_sb[r*8 + i]
                for r in range(2):
                    for i in range(DCH):
                        nc.sync.dma_start(
                            out=v_sb[r * 8 + i][:],
                            in_=agout_v[(r * 8 + i) * P:(r * 8 + i + 1) * P, :])

            # ---- phase 2: attention over four 256-query slots -----------
            # Slots are processed in PAIRS (0,1) and (2,3): for key chunks
            # both slots of a pair need, ONE N=512 matmul covers both
            # (their queries are adjacent in qt).  A pair shares PSUM
            # banks; the upper slot's private chunks continue accumulating
            # into the bank's high half with start=False (has_written bits
            # are already set -- start=True would clear the WHOLE bank and
            # wipe the partner's partials).
            with (
                tc.tile_pool(name="pp", bufs=1) as pp,
                tc.tile_pool(name="ost", bufs=4) as ost,
                tc.tile_pool(name="msc", bufs=2) as msc,
                tc.tile_pool(name="scp", bufs=2, space="PSUM") as scp,
                tc.tile_pool(name="sol", bufs=2, space="PSUM") as sol,
                tc.tile_pool(name="smp", bufs=2, space="PSUM") as smp,
            ):
                TLO = (TPL[0], TPL[2])   # pair g: low slot = 2g, high = 2g+1
                THI = (TPL[1], TPL[3])
                phat = {}   # (g, k) -> [P, 2*QS] (k < TLO) or [P, QS]
                recb = {}   # pair -> [P, 2*QS] f32 broadcast reciprocal

                def pair_sum(g):
                    sps = smp.tile([1, 2 * QS], f32, tag="sm", name=f"sum{g}")
                    for k in range(THI[g]):
                        if k < TLO[g]:
                            nc.tensor.matmul(
                                sps[:], ones_sb[:, 0:1], phat[g, k][:],
                                start=(k == 0), stop=False,
                                skip_group_check=True,
                            )
                        else:
                            nc.tensor.matmul(
                                sps[:, QS:2 * QS], ones_sb[:, 0:1],
                                phat[g, k][:],
                                start=False, stop=(k == THI[g] - 1),
                                skip_group_check=True,
                            )
                    srow = msc.tile([1, 2 * QS], f32, tag="srow", name=f"srow{g}")
                    nc.vector.tensor_copy(srow[:], sps[:])
                    bc = smp.tile([P, 2 * QS], f32, tag="sm", name=f"bc{g}")
                    nc.tensor.matmul(bc[:], onesr[:, 0:P], srow[:],
                                     start=True, stop=True)
                    rb = msc.tile([P, 2 * QS], f32, tag=f"recb{g}",
                                  name=f"recb{g}")
                    nc.vector.reciprocal_approx_fast(out=rb[:], in_=bc[:])
                    recb[g] = rb

                # scores, key-chunk-outer: one kt slice load drives both
                # pairs' matmuls.  Only the last 4 chunks of each slot can
                # need masking (diagonal or template padding); for the low
                # slot those fall in the pair phase (mask the low half),
                # for the high slot in its solo phase.
                for k in range(NKB):
                    ksl = slice(k * P, (k + 1) * P)
                    ps = {}
                    for g in range(2):
                        if k < TLO[g]:
                            ps[g] = scp.tile([P, 2 * QS], f32, tag=f"pg{g}",
                                             name=f"pg{g}_{k}")
                        elif k < THI[g]:
                            ps[g] = sol.tile([P, QS], f32, tag="so",
                                             name=f"so{g}_{k}")
                    for e in range(DCH):
                        for g in range(2):
                            if k < TLO[g]:
                                nc.tensor.matmul(
                                    ps[g][:], kt_sb[e][:, ksl],
                                    qt_sb[e][:, 2 * g * QS:(2 * g + 2) * QS],
                                    start=(e == 0), stop=(e == DCH - 1),
                                    skip_group_check=True,
                                )
                            elif k < THI[g]:
                                nc.tensor.matmul(
                                    ps[g][:], kt_sb[e][:, ksl],
                                    qt_sb[e][:, (2 * g + 1) * QS:
                                               (2 * g + 2) * QS],
                                    start=(e == 0), stop=(e == DCH - 1),
                                    skip_group_check=True,
                                )
                    for g in range(2):
                        if k < TLO[g]:
                            j = k - (TLO[g] - 4)
                            if j >= 0:
                                nc.vector.tensor_tensor(
                                    ps[g][:, 0:QS], ps[g][:, 0:QS],
                                    msk_sb[2 * g][j][:], op=add)
                            ph = pp.tile([P, 2 * QS], bf16, tag=f"pp{g}_{k}",
                                         name=f"php{g}_{k}")
                            nc.scalar.activation(ph[:], ps[g][:], Exp,
                                                 scale=0.03125)
                            phat[g, k] = ph
                        elif k < THI[g]:
                            j = k - (THI[g] - 4)
                            if j >= 0:
                                nc.vector.tensor_tensor(
                                    ps[g][:], ps[g][:],
                                    msk_sb[2 * g + 1][j][:], op=add)
                            ph = pp.tile([P, QS], bf16, tag=f"po{g}_{k}",
                                         name=f"pho{g}_{k}")
                            nc.scalar.activation(ph[:], ps[g][:], Exp,
                                                 scale=0.03125)
                            phat[g, k] = ph
                        if k == THI[g] - 1:
                            pair_sum(g)

                # AV, e-outer / key-chunk-inner: one V slice load drives
                # both pairs.  out^T[e, q], two slots per PSUM bank.
                for e in range(DCH):
                    esl = slice(e * P, (e + 1) * P)
                    av = [scp.tile([P, 2 * QS], f32, tag=f"pg{g}",
                                   name=f"avg{g}_{e}") for g in range(2)]
                    for k in range(NKB):
                        for g in range(2):
                            if k < TLO[g]:
                                nc.tensor.matmul(
                                    av[g][:], v_sb[k][:, esl], phat[g, k][:],
                                    start=(k == 0), stop=False,
                                    skip_group_check=True,
                                )
                            elif k < THI[g]:
                                nc.tensor.matmul(
                                    av[g][:, QS:2 * QS], v_sb[k][:, esl],
                                    phat[g, k][:],
                                    start=False, stop=(k == THI[g] - 1),
                                    skip_group_check=True,
                                )
                    for g in range(2):
                        ot = ost.tile([P, 2 * QS], f32, tag="ot",
                                      name=f"ot{g}_{e}")
                        nc.vector.tensor_tensor(ot[:], av[g][:], recb[g][:],
                                                op=mult)
                        nc.sync.dma_start(
                            out=outT[esl, 2 * g * QS:(2 * g + 2) * QS],
                            in_=ot[:])

    nc.compile()
    return nc


def _make_masks(h: int) -> np.ndarray:
    """[16*128, 256] bf16: slot s rows [4s*128:(4s+4)*128) = key chunks
    TPL[s]-4 .. TPL[s]-1 vs that slot's 256 queries."""
    m = np.empty((16 * P, QS), dtype=np.float32)
    for s in range(4):
        q0 = _QSTARTS[h][s]
        q = q0 + np.arange(QS)[None, :]
        for j in range(4):
            c = TPL[s] - 4 + j
            kk = c * P + np.arange(P)[:, None]
            m[(4 * s + j) * P:(4 * s + j + 1) * P] = np.where(kk <= q, 0.0, NEG)
    return m.astype(_BF16)


def _in_maps(x, Wk, Wq, Wv):
    wq_t = np.ascontiguousarray(Wk.T.astype(_BF16))   # ref swap: q uses Wk
    wk_t = np.ascontiguousarray(Wq.T.astype(_BF16))
    wv_t = np.ascontiguousarray(Wv.T.astype(_BF16))
    mby_h = [_make_masks(0), _make_masks(1)]
    maps = []
    for c in range(8):
        b, h = divmod(c, 2)
        xb = x[b].astype(_BF16)
        # own keys for the K/V projection split
        x_t = np.ascontiguousarray(xb[h * KO:(h + 1) * KO].T)
        xq_t = np.ascontiguousarray(
            np.concatenate([xb[q0:q0 + QS] for q0 in _QSTARTS[h]], axis=0).T
        )
        maps.append({
            "xT": x_t,
            "xqT": xq_t,
            "wqT": wq_t,
            "wkT": wk_t,
            "wvT": wv_t,
            "masks": mby_h[h],
        })
    return maps


def _assemble(results):
    out = np.empty((B, S, D), dtype=np.float32)
    for c, res in enumerate(results):
        b, h = divmod(c, 2)
        o = res["outT"]
        for s, q0 in enumerate(_QSTARTS[h]):
            out[b, q0:q0 + QS] = o[:, s * QS:(s + 1) * QS].T
    return out


def kernel(x, Wk, Wq, Wv, _trace=False):
    from concourse.bass_utils import run_bass_kernel_spmd

    nc = _build_nc()
    res = run_bass_kernel_spmd(nc, _in_maps(x, Wk, Wq, Wv), list(range(8)),
                               trace=_trace)
    out = _assemble(res.results)
    if _trace:
        return out, res
    return out
